# revision 1
# baseline (speedup 1.0000x reference)
"""Trainium2 Bass kernel V3: split-bias FAN-attention w/ dynamic-graph bias.

Data-parallel over batch B=32 across 8 cores (4 batches/core).

Per batch, per head: eT[k,q] = qk-energy + w[q]*dg[q,k], att=exp(eT*scale),
out = vaug.T @ att (ones column gives softmax denominators), then
normalize+project with prepacked matrices.

Engine split (to balance PE/DVE/ACT):
  heads 0,1  (PE path): bias via diag-matmul (fused transpose+scale on PE),
             exp reads 2-bank PSUM units.
  heads 2..7 (DVE path): host ships w*dg pre-multiplied AND pre-transposed
             [k,q]; DVE adds it onto the energy PSUM writing z (fp16) to
             SBUF; exp reads SBUF in [128,4096] chunks (8 tiles per chunk).
Host precomputes FAN projections q/k/v and gates w1/w2 (tiny vs dg traffic).
"""
import numpy as np

B, N, E, H, D = 32, 512, 40, 8, 5
NCORES = 8
B_LOC = B // NCORES
SCALE = 1.0 / float(np.float32(E) ** 0.5)
CHN = 96
CHBASE = [0, 5, 32, 37, 64, 69, 74, 79]
GBASE = [0, 0, 32, 32, 64, 64, 64, 64]
QCOL = [0, 512, 0, 512, 0, 512, 1024, 1536]

_PROG_CACHE = {}
_U_TILES = 2  # PSUM-exp unit size for PE-path heads (3-bank tiles break HW)


def _build_program(reps=1):
    key = f"nc{reps}_u{_U_TILES}"
    if key in _PROG_CACHE:
        return _PROG_CACHE[key]
    import contextlib
    import concourse.bass as bass
    import concourse.tile as tile
    from concourse import bacc, mybir

    F32 = mybir.dt.float32
    BF16 = mybir.dt.bfloat16
    FP16 = mybir.dt.float16
    AF = mybir.ActivationFunctionType
    OP = mybir.AluOpType

    nc = bacc.Bacc(None)
    dp = nc.declare_dram_parameter
    kt_d = dp("kt", [B_LOC, CHN, N], BF16, isOutput=False)
    qtm_d = dp("qtm", [B_LOC, CHN, 2048], BF16, isOutput=False)
    va_d = dp("va", [B_LOC, 128, 192], BF16, isOutput=False)
    dw_d = dp("dw", [B_LOC, 128, 512], BF16, isOutput=False)
    dgn_d = dp("dgn", [B_LOC, 3, 128, 4 * N], BF16, isOutput=False)
    dgw_d = dp("dgw", [B_LOC, 5, 128, 4 * N], BF16, isOutput=False)
    sel_lo_d = dp("sel_lo", [128, 8], BF16, isOutput=False)
    sel_hi_d = dp("sel_hi", [128, 8], BF16, isOutput=False)
    e5_lo_d = dp("e5_lo", [8, 128], BF16, isOutput=False)
    e5_hi_d = dp("e5_hi", [8, 128], BF16, isOutput=False)
    p_lo_d = dp("p_lo", [128, E], BF16, isOutput=False)
    p_hi_d = dp("p_hi", [128, E], BF16, isOutput=False)
    projb_d = dp("projb", [E, 1], F32, isOutput=False)
    out_d = dp("outT", [B_LOC, E, N], F32, isOutput=True)

    lp = nc.allow_low_precision(reason="bf16 datapath validated vs reference")
    lp.__enter__()
    with tile.TileContext(nc) as tc:
        with (
            tc.tile_pool(name="const", bufs=1) as cp,
            tc.tile_pool(name="inp", bufs=2) as ip,
            tc.tile_pool(name="dgnp", bufs=3) as dgnp,
            tc.tile_pool(name="dgwp", bufs=8) as dgwp,
            tc.tile_pool(name="attp", bufs=3) as attp,
            tc.tile_pool(name="zp", bufs=2) as zp,
            tc.tile_pool(name="wrk", bufs=2) as wp,
            tc.tile_pool(name="ps", bufs=3, space=bass.MemorySpace.PSUM) as ps,
        ):
            def cload(dram, shape, tag, dt=BF16):
                t = cp.tile(shape, dt, tag=tag, name=tag)
                nc.sync.dma_start(t[:], dram[:])
                return t

            sel_lo = cload(sel_lo_d, [128, 8], "sel_lo")
            sel_hi = cload(sel_hi_d, [128, 8], "sel_hi")
            e5_lo = cload(e5_lo_d, [8, 128], "e5_lo")
            e5_hi = cload(e5_hi_d, [8, 128], "e5_hi")
            p_lo = cload(p_lo_d, [128, E], "p_lo")
            p_hi = cload(p_hi_d, [128, E], "p_hi")
            projb = cload(projb_d, [E, 1], "projb", F32)

            U = _U_TILES

            loop_ctx = tc.For_i(0, reps) if reps > 1 else contextlib.nullcontext()
            with loop_ctx:
                pend = None
                stage5 = None
                for b in range(B_LOC):
                    kt = ip.tile([CHN, N], BF16, tag="kt", name="kt")
                    nc.sync.dma_start(kt[:], kt_d[b])
                    qtm = ip.tile([CHN, 2048], BF16, tag="qtm", name="qtm")
                    nc.sync.dma_start(qtm[:], qtm_d[b])
                    va = ip.tile([128, 192], BF16, tag="va", name="va")
                    nc.sync.dma_start(va[:], va_d[b])
                    dw = ip.tile([128, 512], BF16, tag="dw", name="dw")
                    nc.sync.dma_start(dw[:], dw_d[b])
                    dgn = []
                    for m in range(3):
                        t = dgnp.tile([128, 2048], BF16, tag="dgn", name="dgn")
                        nc.sync.dma_start(t[:], dgn_d[b, m])
                        dgn.append(t)
                    dgw = []
                    for m in range(5):
                        t = dgwp.tile([128, 2048], BF16, tag="dgw", name="dgw")
                        nc.gpsimd.dma_start(t[:], dgw_d[b, m])
                        dgw.append(t)

                    out_lo = ps.tile([128, N], F32, tag="out_lo", bufs=2,
                                     name="out_lo")
                    out_hi = ps.tile([128, N], F32, tag="out_hi", bufs=2,
                                     name="out_hi")
                    nc.vector.memset(out_lo[:], 0.0)
                    nc.vector.memset(out_hi[:], 0.0)

                    class Regs:
                        """Rotating [128,512] psum regions from the unit tag."""
                        def __init__(self):
                            self.cur, self.idx = None, U

                        def get(self, nm):
                            if self.idx >= U:
                                self.cur = ps.tile([128, 512 * U], F32,
                                                   tag="unit", name=nm,
                                                   bufs=4 // U)
                                self.idx = 0
                            r = self.cur[:, 512 * self.idx:
                                         512 * (self.idx + 1)]
                            self.idx += 1
                            return r

                    regs = Regs()

                    def stage5a(bb, o_lo, o_hi, st):
                        sb = wp.tile([128, 1024], BF16, tag="sb", name="sb")
                        st["sb"] = sb
                        nc.scalar.copy(sb[:, 0:N], o_lo[:])
                        nc.scalar.copy(sb[:, N:2 * N], o_hi[:])
                        n1 = regs.get("n1")
                        st["n1"] = n1
                        nc.tensor.matmul(n1[0:8, :], sel_lo[:], sb[:, 0:N],
                                         start=True, stop=False)
                        nc.tensor.matmul(n1[0:8, :], sel_hi[:], sb[:, N:2 * N],
                                         start=False, stop=True)
                        recip8 = wp.tile([8, N], BF16, tag="recip8",
                                         name="recip8")
                        st["recip8"] = recip8
                        nc.vector.reciprocal(recip8[:], n1[0:8, :])

                    def stage5b(bb, o_lo, o_hi, st):
                        regs.idx = U  # fresh tile so rm_lo/rm_hi are adjacent
                        rm_lo = regs.get("rm_lo")
                        rm_hi = regs.get("rm_hi")
                        st["rm"] = regs.cur
                        nc.tensor.matmul(rm_lo[:], e5_lo[:], st["recip8"][:],
                                         start=True, stop=True)
                        nc.tensor.matmul(rm_hi[:], e5_hi[:], st["recip8"][:],
                                         start=True, stop=True)
                        sbn = wp.tile([128, 1024], BF16, tag="sbn", name="sbn")
                        st["sbn"] = sbn
                        nc.vector.tensor_tensor(sbn[:], st["sb"][:],
                                                st["rm"][:, 0:1024],
                                                op=OP.mult)

                    def stage5c(bb, o_lo, o_hi, st):
                        n2 = regs.get("n2")
                        nc.tensor.matmul(n2[0:E, :], p_lo[:],
                                         st["sbn"][:, 0:N],
                                         start=True, stop=False)
                        nc.tensor.matmul(n2[0:E, :], p_hi[:],
                                         st["sbn"][:, N:2 * N],
                                         start=False, stop=True)
                        out_sb = wp.tile([E, N], F32, tag="out_sb",
                                         name="out_sb")
                        nc.scalar.activation(out_sb[:], n2[0:E, :],
                                             AF.Identity, bias=projb[:])
                        nc.gpsimd.dma_start(out_d[bb][:], out_sb[:])

                    def energy(ue, h, j, start, stop):
                        g, qc = GBASE[h], QCOL[h]
                        nc.tensor.matmul(
                            ue, kt[g:g + 32, 128 * j:128 * (j + 1)],
                            qtm[g:g + 32, qc:qc + N],
                            start=start, stop=stop, skip_group_check=True)

                    def out_mm(h, j, att_slice):
                        hh = h % 4
                        obase = 32 * hh
                        ps_out = out_lo if h < 4 else out_hi
                        nc.tensor.matmul(
                            ps_out[obase:obase + 6, :],
                            va[:, 48 * j + 6 * h:48 * j + 6 * h + 6],
                            att_slice,
                            start=(j == 0), stop=(j == 3),
                            tile_position=(0, obase),
                            skip_group_check=True)

                    # ---- PE path: heads 0,1 -> 8 tiles in units of U ----
                    p_tiles = [(h, j) for h in (0, 1, 2) for j in range(4)]
                    p_units = [p_tiles[i:i + U] for i in range(0, len(p_tiles), U)]
                    patt = []

                    def p_fill(u):
                        ue = ps.tile([128, 512 * U], F32, tag="unit",
                                     name="pue", bufs=4 // U)
                        for idx, (h, j) in enumerate(p_units[u]):
                            col = idx * 512
                            energy(ue[:, col:col + 512], h, j, True, True)
                            for i in range(4):
                                nc.tensor.matmul(
                                    ue[:, col + 128 * i:col + 128 * (i + 1)],
                                    dgn[h][:, 512 * i + 128 * j:
                                           512 * i + 128 * (j + 1)],
                                    dw[:, 128 * i:128 * (i + 1)],
                                    start=False, stop=True,
                                    skip_group_check=True)
                        at = attp.tile([128, 512 * U], BF16, tag="attp",
                                       name="pat", bufs=8 // U)
                        nc.scalar.activation(at[:], ue[:], AF.Exp, scale=SCALE)
                        patt.append(at)

                    def p_outs(u):
                        for idx, (h, j) in enumerate(p_units[u]):
                            out_mm(h, j, patt[u][:, idx * 512:(idx + 1) * 512])

                    # ---- DVE path: heads 2..7 in chunks of 2 heads ----
                    d_chunks = [(3, 4), (5, 6), (7,)]
                    datt = []

                    def d_pair(zb, pos, h, j):
                        regs.idx = U  # fresh unit tile so the pair is adjacent
                        ue0 = regs.get("due")
                        energy(ue0, h, j, True, True)
                        ue1 = regs.get("due")
                        energy(ue1, h, j + 1, True, True)
                        nc.vector.tensor_tensor(
                            zb[:, 512 * pos:512 * (pos + 2)],
                            regs.cur[:, 0:1024],
                            dgw[h - 3][:, 512 * j:512 * (j + 2)],
                            op=OP.add)

                    def d_exp(zb, w):
                        at = attp.tile([128, 4096], BF16, tag="attd",
                                       name="dat")
                        nc.scalar.activation(at[:, 0:w], zb[:, 0:w],
                                             AF.Exp, scale=SCALE)
                        datt.append(at)

                    def d_outs(c):
                        for pos, (h, j) in enumerate(
                                (h, j) for h in d_chunks[c] for j in range(4)):
                            out_mm(h, j, datt[c][:, 512 * pos:512 * (pos + 1)])


                    # ---- emission schedule (pipelined) ----
                    csize = [4 * len(hc) * 512 for hc in d_chunks]
                    st = {}
                    nu = len(p_units)
                    # flatten D work into (chunk, pos, h, j) pairs
                    pairs = []
                    for c, hc in enumerate(d_chunks):
                        tiles = [(h, j) for h in hc for j in range(4)]
                        for p0 in range(0, len(tiles), 2):
                            pairs.append((c, p0) + tiles[p0])
                    zbs = {}
                    p_idx = 0
                    for i, (c, pos, h, j) in enumerate(pairs):
                        if c not in zbs:
                            zbs[c] = zp.tile([128, 4096], FP16, tag="z",
                                             name="zb")
                        d_pair(zbs[c], pos, h, j)
                        last_of_chunk = (i + 1 == len(pairs)
                                         or pairs[i + 1][0] != c)
                        if last_of_chunk:
                            d_exp(zbs[c], csize[c])
                        if i % 2 == 0 and p_idx < nu:
                            p_fill(p_idx)
                            p_idx += 1
                    while p_idx < nu:
                        p_fill(p_idx)
                        p_idx += 1
                    if pend is not None:
                        stage5a(*pend, st)
                    for u in range(nu):
                        p_outs(u)
                    if pend is not None:
                        stage5b(*pend, st)
                    d_outs(0)
                    if pend is not None:
                        stage5c(*pend, st)
                    d_outs(1)
                    d_outs(2)
                    pend = (b, out_lo, out_hi)
                st = {}
                stage5a(*pend, st)
                stage5b(*pend, st)
                stage5c(*pend, st)

    lp.__exit__(None, None, None)
    nc.compile()
    _PROG_CACHE[key] = nc
    return nc


def _host_arrays(inputs):
    import ml_dtypes
    bf16 = ml_dtypes.bfloat16
    f32 = np.float32
    x = np.asarray(inputs["x"], f32)

    def fan(p):
        ph = x @ inputs[f"{p}_Wp"] + inputs[f"{p}_bp"]
        g = x @ inputs[f"{p}_Wg"] + inputs[f"{p}_bg"]
        return np.concatenate([np.cos(ph), np.sin(ph), g], -1)  # (B,N,40)

    q, k, v = fan("q"), fan("k"), fan("v")
    w1 = 1.0 / (1.0 + np.exp(-(q[:, :, :20] @ inputs["dg1_W"]
                               + inputs["dg1_b"])))[..., 0]  # (B,N)
    w2 = 1.0 / (1.0 + np.exp(-(q[:, :, 20:] @ inputs["dg2_W"]
                               + inputs["dg2_b"])))[..., 0]

    kT = k.transpose(0, 2, 1)  # (B,40,N)
    ktp = np.zeros((B, CHN, N), f32)
    ktp[:, 0:10] = kT[:, 0:10]
    ktp[:, 32:42] = kT[:, 10:20]
    ktp[:, 64:84] = kT[:, 20:40]
    qT = q.transpose(0, 2, 1)
    qtm = np.zeros((B, CHN, 2048), f32)
    for h in range(H):
        base, qc = CHBASE[h], QCOL[h]
        qtm[:, base:base + 5, qc:qc + N] = qT[:, 5 * h:5 * h + 5]

    va = np.ones((B, 4, 128, 48), f32)
    vr = v.reshape(B, 4, 128, 8, 5)
    for h in range(H):
        va[:, :, :, 6 * h:6 * h + 5] = vr[:, :, :, h, :]
    va = np.ascontiguousarray(va.transpose(0, 2, 1, 3)).reshape(B, 128, 192)

    # diag blocks for PE-path (w1, q-chunks 0..3)
    dw = np.zeros((B, 128, 512), f32)
    idx = np.arange(128)
    for i in range(4):
        dw[:, idx, 128 * i + idx] = w1[:, 128 * i + idx]

    sel_lo = np.zeros((128, 8), f32)
    sel_hi = np.zeros((128, 8), f32)
    e5_lo = np.zeros((8, 128), f32)
    e5_hi = np.zeros((8, 128), f32)
    p_lo = np.zeros((128, E), f32)
    p_hi = np.zeros((128, E), f32)
    for hh in range(4):
        sel_lo[32 * hh + 5, hh] = 1.0
        sel_hi[32 * hh + 5, 4 + hh] = 1.0
        for j in range(5):
            e5_lo[hh, 32 * hh + j] = 1.0
            e5_hi[4 + hh, 32 * hh + j] = 1.0
            p_lo[32 * hh + j, :] = inputs["proj_W"][5 * hh + j, :]
            p_hi[32 * hh + j, :] = inputs["proj_W"][20 + 5 * hh + j, :]

    consts = dict(
        sel_lo=sel_lo.astype(bf16), sel_hi=sel_hi.astype(bf16),
        e5_lo=e5_lo.astype(bf16), e5_hi=e5_hi.astype(bf16),
        p_lo=p_lo.astype(bf16), p_hi=p_hi.astype(bf16),
        projb=np.ascontiguousarray(
            np.asarray(inputs["proj_b"], f32).reshape(E, 1)))

    dg1 = np.asarray(inputs["dynamic_graph1"], f32)  # (B,4,N,N) [q,k]
    dg2 = np.asarray(inputs["dynamic_graph2"], f32)

    # PE-path: natural layout, heads 0,1 (dg1 hh 0,1):
    # tile[p, 512c+x] = dg[b,hh,128c+p,x]
    dgn = np.ascontiguousarray(
        dg1[:, 0:3].reshape(B, 3, 4, 128, N).transpose(0, 1, 3, 2, 4)
    ).reshape(B, 3, 128, 4 * N).astype(bf16)

    # DVE-path: premultiplied by w, transposed to [k,q]:
    # tile[p, 512j+q] = w[b,q]*dg[b,hh,q,128j+p]
    def dgw_pack(dg, w, hh_list):
        out = np.empty((B, len(hh_list), 128, 4 * N), np.float32)
        for m, hh in enumerate(hh_list):
            p = dg[:, hh] * w[:, :, None]            # (B, q, k)
            pt = p.transpose(0, 2, 1)                # (B, k, q)
            out[:, m] = pt.reshape(B, 4, 128, N).transpose(
                0, 2, 1, 3).reshape(B, 128, 4 * N)
        return out.astype(bf16)

    dgw = np.concatenate([dgw_pack(dg1, w1, [3]),
                          dgw_pack(dg2, w2, [0, 1, 2, 3])], axis=1)

    per_batch = dict(kt=ktp.astype(bf16), qtm=qtm.astype(bf16),
                     va=va.astype(bf16), dw=dw.astype(bf16),
                     dgn=dgn, dgw=dgw)
    return per_batch, consts


def _make_in_maps(inputs):
    per_batch, consts = _host_arrays(inputs)
    in_maps = []
    for c in range(NCORES):
        sl = slice(c * B_LOC, (c + 1) * B_LOC)
        m = {k: np.ascontiguousarray(v[sl]) for k, v in per_batch.items()}
        m.update(consts)
        in_maps.append(m)
    return in_maps


def kernel(**inputs):
    from concourse.bass_utils import run_bass_kernel_spmd

    nc = _build_program()
    in_maps = _make_in_maps(inputs)
    res = run_bass_kernel_spmd(nc, in_maps, list(range(NCORES)))
    outT = np.concatenate([res.results[c]["outT"] for c in range(NCORES)], 0)
    return np.ascontiguousarray(outT.transpose(0, 2, 1)).astype(np.float32)

